# revision 95
# baseline (speedup 1.0000x reference)
"""MultiHeadAttention (partial RoPE) Trainium2 Bass kernel.

Sharding: 8 cores = 2 batches x 4 head-groups (4 heads each).
Each core computes a partial output (L, D) for its batch from its 4 heads;
the host sums the 4 partials per batch (the "all-reduce after o-proj").

Per-core dataflow (transposed layout; no on-chip transposes):
  phase 1: k/v projections + q for the first lq tile.
    q/k produced as two [128, L] column-groups (one per head PAIR g):
    rows s*64+t = head (2g+s) dim t, t<32 rot / t>=32 pass.  This
    interleaved layout makes QK^T a contraction-64 matmul (2 heads
    packed in the PE array as 64-row strips) instead of two
    contraction-32 passes.  RoPE: the "a" projection lands rot+pass
    in-layout; rotate_half is a separate host-composed weight group;
    qc = ps_a*cos (cos table carries 1.0 at pass rows), then
    qc += ps_h*sin on the 32 rot rows of each head (sin table in qc
    row layout; PSUM operands are exempt from the engine rule that all
    SBUF operands share a start partition).
    v produced as [L, hk] with a ones column per head (65 cols) so the
    AV matmul also emits softmax row-sums for free.
  phase 2 (everything else, one pipelined loop over lq tiles):
    per lq-tile(512) x lk-tile(128):
      QK^T: 4 matmuls (contract 64, tile_position strips) -> 2x
            [128,2,512] PSUM; AV matmuls are emitted one lk behind so
            the in-order PE queue never head-blocks on exp.
      exp:  head-pair 0 on ACT; head-pair 1 alternates between ACT and
            [DVE copy to SBUF + exact gpsimd pow-ucode] so no single
            engine paces the loop.
      AV:   per head, [v_h|1] lhsT (M=65) accumulating over lk.
    per lq tile, overlapped with the next one:
      normalize/evict: reciprocal of the av sum row, gpsimd
      partition_broadcast, one fused mul per head straight from PSUM
      into attn2 (head pairs stacked on partitions).
      o-projection of the previous lq tile and q-projection of the
      next one ride the same qk PSUM tag ring (no extra banks), fill
      the PE slack, and stream bf16 partials out via rotating DMA
      queues.
"""

import sys

if "/opt/trn_rl_repo" not in sys.path:
    sys.path.insert(0, "/opt/trn_rl_repo")

from contextlib import ExitStack

import ml_dtypes
import numpy as np

import concourse.bass as bass
import concourse.mybir as mybir
from concourse import bacc
import concourse.tile as tile

B, L_FULL, D = 2, 2048, 1024
H, K = 16, 64
ROT = 32
HPC = 4  # heads per core
NCORES = 8

F32 = mybir.dt.float32
BF16 = mybir.dt.bfloat16
NPBF = ml_dtypes.bfloat16

LQ = 512  # lq tile (matmul N)
LK = 128  # lk tile (matmul M / partition tile)
NC = D // 128  # contraction chunks for projections
POOL_EXP = True  # offload head-pair 1 exp to the gpsimd pow ucode



def build_nc(L=L_FULL):
    """Build the single-core SPMD program. Returns nc."""
    nc = bacc.Bacc("TRN2", target_bir_lowering=False)

    NLQ = L // LQ
    NLK = L // LK
    NLT = L // LQ

    # ---- DRAM I/O (per-core shapes, host pre-swizzled) ----
    xq = nc.dram_tensor("xq", [128, NC, L], BF16, kind="ExternalInput")
    xk = nc.dram_tensor("xk", [128, NC, L], BF16, kind="ExternalInput")
    xv = nc.dram_tensor("xv", [128, NC, L], BF16, kind="ExternalInput")
    wqa0 = nc.dram_tensor("wqa0", [128, NC, 128], BF16, kind="ExternalInput")
    wqa1 = nc.dram_tensor("wqa1", [128, NC, 128], BF16, kind="ExternalInput")
    wqh = nc.dram_tensor("wqh", [128, NC, 128], BF16, kind="ExternalInput")
    wka0 = nc.dram_tensor("wka0", [128, NC, 128], BF16, kind="ExternalInput")
    wka1 = nc.dram_tensor("wka1", [128, NC, 128], BF16, kind="ExternalInput")
    wkh = nc.dram_tensor("wkh", [128, NC, 128], BF16, kind="ExternalInput")
    wv = nc.dram_tensor("wv", [128, NC, 256], BF16, kind="ExternalInput")
    wo2 = nc.dram_tensor("wo2", [2, 128, D], BF16, kind="ExternalInput")
    cosq = nc.dram_tensor("cosq", [128, L], F32, kind="ExternalInput")
    sinr = nc.dram_tensor("sinr", [128, L], F32, kind="ExternalInput")
    # bf16 partials: host accumulates the 4 head-group partials in f32
    out = nc.dram_tensor("out", [L, D], BF16, kind="ExternalOutput")

    with tile.TileContext(nc) as tc, ExitStack() as ctx:
        consts = ctx.enter_context(tc.tile_pool(name="consts", bufs=1))
        persist = ctx.enter_context(tc.tile_pool(name="persist", bufs=1))

        # ---- weights / trig tables: gpsimd queue (SP stays free for x) ----
        def wtile(dram, ncols):
            t = consts.tile([128, NC, ncols], BF16, tag=f"w_{dram.name}")
            nc.gpsimd.dma_start(out=t[:], in_=dram[:])
            return t

        wka0_s, wka1_s, wkh_s = wtile(wka0, 128), wtile(wka1, 128), wtile(wkh, 128)
        cos_s = consts.tile([128, L], F32, tag="cos")
        sin_s = consts.tile([128, L], F32, tag="sin")
        nc.gpsimd.dma_start(out=cos_s[:], in_=cosq[:])
        nc.gpsimd.dma_start(out=sin_s[:], in_=sinr[:])
        wv_s = wtile(wv, 256)
        wqa0_s, wqa1_s, wqh_s = wtile(wqa0, 128), wtile(wqa1, 128), wtile(wqh, 128)
        wo2_s = consts.tile([128, 2, D], BF16, tag="wo2")
        nc.gpsimd.dma_start(out=wo2_s[:], in_=wo2.rearrange("g p e -> p g e"))

        # ---- persistent activations ----
        qc = persist.tile([128, 2, L], BF16, tag="qc")
        kc = persist.tile([128, 2, L], BF16, tag="kc")
        v_s = persist.tile([128, NLK, HPC, 65], BF16, tag="v")
        attn2 = persist.tile([128, 2, L], BF16, tag="attn2")

        nc.vector.memset(v_s[:, :, :, 64:65], 1.0)

        xpool = ctx.enter_context(tc.tile_pool(name="xpool", bufs=2))
        tpool = ctx.enter_context(tc.tile_pool(name="tpool", bufs=2))

        def rope_evict(dst, ls, ps_a0, ps_a1, ps_h):
            # SBUF operands of an engine op must share a start partition;
            # PSUM operands are exempt.  So the rot products are written
            # straight into qc-layout rows (out + sin table aligned), with
            # ps_h read at its own (32-aligned) PSUM rows.
            t2s = tpool.tile([128, 2, LQ], F32, tag="t2s")
            nc.vector.tensor_mul(dst[:, 0, ls], ps_a0[:], cos_s[:, ls])
            nc.vector.tensor_mul(dst[:, 1, ls], ps_a1[:], cos_s[:, ls])
            for g in (0, 1):
                for s in (0, 1):
                    h = 2 * g + s
                    rs = slice(s * 64, s * 64 + 32)
                    nc.vector.tensor_mul(
                        t2s[rs, g, :],
                        ps_h[h * 32 : (h + 1) * 32, :],
                        sin_s[rs, ls],
                    )
            for s in (0, 1):
                rs = slice(s * 64, s * 64 + 32)
                nc.vector.tensor_add(dst[rs, :, ls], dst[rs, :, ls], t2s[rs, :, :])

        def load_x(dram, tag, ls, split=False, eng=None):
            eng = eng or nc.sync
            x_s = xpool.tile([128, NC, LQ], BF16, tag=tag)
            if split:  # first chunk lands sooner so matmuls start earlier
                eng.dma_start(out=x_s[:, 0:2, :], in_=dram[:, 0:2, ls])
                eng.dma_start(out=x_s[:, 2:NC, :], in_=dram[:, 2:NC, ls])
            else:
                eng.dma_start(out=x_s[:], in_=dram[:, :, ls])
            return x_s

        # ===== phase 1: k/v projections (+ q for the first lq tile) =====
        # q projections for lq tiles 1..3 are folded into the attention loop
        with tc.tile_pool(name="pps", bufs=2, space="PSUM") as pps:
            xks = [load_x(xk, "xk", slice(0, LQ), split=True)]
            for lt in range(NLT):
                ls = slice(lt * LQ, (lt + 1) * LQ)
                x_k = xks[lt]
                if lt + 1 < NLT:
                    xks.append(
                        load_x(xk, "xk", slice((lt + 1) * LQ, (lt + 2) * LQ))
                    )
                x_v = load_x(xv, "xv", ls)
                if lt == 0:
                    x_q0 = load_x(xq, "xq", ls, eng=nc.gpsimd)

                ps_a0 = pps.tile([128, LQ], F32, tag="ppa")
                ps_a1 = pps.tile([128, LQ], F32, tag="ppa")
                ps_h = pps.tile([128, LQ], F32, tag="pph")
                for ps, w in ((ps_a0, wka0_s), (ps_a1, wka1_s), (ps_h, wkh_s)):
                    for c in range(NC):
                        nc.tensor.matmul(
                            ps[:],
                            w[:, c, :],
                            x_k[:, c, :],
                            start=(c == 0),
                            stop=(c == NC - 1),
                        )
                rope_evict(kc, ls, ps_a0, ps_a1, ps_h)

                if lt == NLT - 2:
                    # q projection for lq 0: before the last v block so its
                    # RoPE tail overlaps the v matmuls
                    ls0 = slice(0, LQ)
                    qs_a0 = pps.tile([128, LQ], F32, tag="ppa")
                    qs_a1 = pps.tile([128, LQ], F32, tag="ppa")
                    qs_h = pps.tile([128, LQ], F32, tag="pph")
                    for ps, w in ((qs_a0, wqa0_s), (qs_a1, wqa1_s), (qs_h, wqh_s)):
                        for c in range(NC):
                            nc.tensor.matmul(
                                ps[:],
                                w[:, c, :],
                                x_q0[:, c, :],
                                start=(c == 0),
                                stop=(c == NC - 1),
                            )
                    rope_evict(qc, ls0, qs_a0, qs_a1, qs_h)

                for st in range(LQ // LK):
                    lk_i = lt * (LQ // LK) + st
                    ps_v = pps.tile([128, 256], F32, tag="ppv")
                    for c in range(NC):
                        nc.tensor.matmul(
                            ps_v[:],
                            x_v[:, c, st * LK : (st + 1) * LK],
                            wv_s[:, c, :],
                            start=(c == 0),
                            stop=(c == NC - 1),
                        )
                    nc.scalar.activation(
                        out=v_s[:, lk_i, :, 0:64],
                        in_=ps_v.rearrange("p (h d) -> p h d", h=HPC),
                        func=mybir.ActivationFunctionType.Copy,
                    )


        # ========== phase 2: attention + interleaved o-projection ==========
        # the o-projection of lq-1 is folded into lq's lk loop, allocating
        # its PSUM tiles from the same qk tag ring (same shape, no extra
        # banks); its matmuls fill the PE slack left by the ACT-paced exp
        with tc.tile_pool(name="qkps", bufs=1, space="PSUM") as qkpool, tc.tile_pool(
            name="avps", bufs=1, space="PSUM"
        ) as avpool, tc.tile_pool(name="ut", bufs=3) as utpool, tc.tile_pool(
            name="npool", bufs=2
        ) as npool, tc.tile_pool(name="oev", bufs=4) as oev:
            oq = [nc.sync, nc.gpsimd]

            def emit_oproj(lt, slot, last=False):
                # one 128-row l-chunk of the o-projection, PSUM via qk ring
                lts = slice(lt * LK, (lt + 1) * LK)
                po = qkpool.tile(
                    [128, 2, LQ], F32, tag=f"qk{slot % 2}", name="po"
                )
                for eh in (0, 1):
                    es = slice(eh * LQ, (eh + 1) * LQ)
                    for g in (0, 1):
                        nc.tensor.matmul(
                            po[:, eh, :],
                            attn2[:, g, lts],
                            wo2_s[:, g, es],
                            start=(g == 0),
                            stop=(g == 1),
                        )
                ot = oev.tile([128, 2, LQ], BF16, tag="ot")
                nc.scalar.activation(
                    out=ot[:],
                    in_=po[:],
                    func=mybir.ActivationFunctionType.Copy,
                )
                dq = nc.sync if last else oq[lt % 2]
                dq.dma_start(out=out[lts, :], in_=ot[:])

            def emit_qproj(lt, part, x_s):
                # q projection for the NEXT lq tile, PSUM via the qk ring:
                # part 0 = both "a" groups, part 1 = the rotate-half group
                ls = slice(lt * LQ, (lt + 1) * LQ)
                ps = qkpool.tile(
                    [128, 2, LQ], F32, tag=f"qk{part}", name="qproj"
                )
                ws = (wqa0_s, wqa1_s) if part == 0 else (wqh_s,)
                for i, w in enumerate(ws):
                    for c in range(NC):
                        nc.tensor.matmul(
                            ps[:, i, :],
                            w[:, c, :],
                            x_s[:, c, :],
                            start=(c == 0),
                            stop=(c == NC - 1),
                        )
                if part == 0:
                    nc.vector.tensor_mul(qc[:, 0, ls], ps[:, 0, :], cos_s[:, ls])
                    nc.vector.tensor_mul(qc[:, 1, ls], ps[:, 1, :], cos_s[:, ls])
                else:
                    t2s = tpool.tile([128, 2, LQ], F32, tag="t2s")
                    for g in (0, 1):
                        for s in (0, 1):
                            h = 2 * g + s
                            rs = slice(s * 64, s * 64 + 32)
                            nc.vector.tensor_mul(
                                t2s[rs, g, :],
                                ps[h * 32 : (h + 1) * 32, 0, :],
                                sin_s[rs, ls],
                            )
                    for s in (0, 1):
                        rs = slice(s * 64, s * 64 + 32)
                        nc.gpsimd.tensor_add(
                            qc[rs, :, ls], qc[rs, :, ls], t2s[rs, :, :]
                        )

            ebase = consts.tile([128, LQ], F32, tag="ebase")
            nc.vector.memset(ebase[:], float(np.exp(1.0 / np.sqrt(K))))
            for lq in range(NLQ):
                qs = slice(lq * LQ, (lq + 1) * LQ)
                av = [
                    avpool.tile([65, LQ], F32, tag=f"av{j}", name=f"av{j}")
                    for j in range(HPC)
                ]

                def emit_exp(qk, ut, g, lk):
                    if g == 0 or not POOL_EXP or lk % 2 == 0:
                        nc.scalar.activation(
                            out=ut[:],
                            in_=qk[:],
                            func=mybir.ActivationFunctionType.Exp,
                            scale=float(1.0 / np.sqrt(K)),
                        )
                        return
                    # head-pair 1, alternate lks: copied to SBUF by DVE
                    # (gpsimd can't read PSUM), exponentiated by the Pool
                    # pow-ucode (exact, ebase^logit = exp(logit/8))
                    qksb = utpool.tile([128, 2, LQ], F32, tag="qksb")
                    nc.vector.tensor_copy(out=qksb[:], in_=qk[:])
                    for s in (0, 1):
                        nc.gpsimd.tensor_tensor(
                            out=ut[:, s, :],
                            in0=ebase[:],
                            in1=qksb[:, s, :],
                            op=mybir.AluOpType.pow,
                        )

                def emit_av(uts, lk):
                    for g in (0, 1):
                        for s in (0, 1):
                            j = 2 * g + s
                            nc.tensor.matmul(
                                av[j][:],
                                v_s[:, lk, j, :],
                                uts[g][:, s, :],
                                start=(lk == 0),
                                stop=(lk == NLK - 1),
                            )

                prev = None
                for lk in range(NLK):
                    ks = slice(lk * LK, (lk + 1) * LK)
                    qks = []
                    for g in (0, 1):
                        qk = qkpool.tile(
                            [128, 2, LQ], F32, tag=f"qk{g}", name=f"qk{g}"
                        )
                        for s in (0, 1):
                            rs = slice(s * 64, (s + 1) * 64)
                            nc.tensor.matmul(
                                qk[:, s, :], kc[rs, g, ks], qc[rs, g, qs],
                                start=True, stop=True,
                            )
                        qks.append(qk)
                    # av matmuls for the PREVIOUS lk: their exp results are
                    # long done, so the PE queue never head-blocks on ACT
                    if prev is not None:
                        emit_av(prev[0], prev[1])
                    uts = []
                    for g in (0, 1):
                        ut = utpool.tile([128, 2, LQ], BF16, tag=f"ut{g}")
                        emit_exp(qks[g], ut, g, lk)
                        uts.append(ut)
                    prev = (uts, lk)
                    # o-projection of the previous lq tile, one l-chunk per
                    # 4 lk iterations (lk 3/7/11/15)
                    if lq > 0 and lk % 4 == 3:
                        emit_oproj(4 * (lq - 1) + lk // 4, lk // 4)
                    # q projection for the next lq tile
                    if lq < NLQ - 1:
                        if lk == 1:
                            x_qn = load_x(xq, "xq", slice((lq + 1) * LQ, (lq + 2) * LQ))
                        elif lk == 5:
                            emit_qproj(lq + 1, 0, x_qn)
                        elif lk == 9:
                            emit_qproj(lq + 1, 1, x_qn)
                emit_av(prev[0], prev[1])
                # normalize while evicting: recip row, SBUF->SBUF broadcast
                # DMA (stride-0 partition source), then one fused
                # normalize-mul per head straight from PSUM into attn2
                # (av is PSUM: exempt from the SBUF partition-match rule)
                recs, rbs = [], []
                for j in range(HPC):
                    rec = npool.tile([1, LQ], F32, tag="rec")
                    nc.vector.reciprocal(out=rec[:], in_=av[j][64:65, :])
                    recs.append(rec)
                for j in range(HPC):
                    rb = npool.tile([128, LQ], F32, tag="rb")
                    nc.gpsimd.partition_broadcast(rb[:], recs[j][0:1, :], channels=128)
                    rbs.append(rb)
                for j in range(HPC):
                    g, s = divmod(j, 2)
                    rs = slice(s * 64, (s + 1) * 64)
                    nc.vector.tensor_mul(
                        attn2[rs, g, qs], av[j][0:64, :], rbs[j][rs, :]
                    )

            # o-projection of the last lq tile
            for i in range(4):
                emit_oproj(4 * (NLQ - 1) + i, i, last=True)

    nc.compile()
    return nc


# ---------------- host side ----------------


def _rot_half_weights(w):
    # w: (D, 32) rot-dim columns for one head -> rotate_half-composed weights
    rh = np.empty_like(w)
    rh[:, 0::2] = -w[:, 1::2]
    rh[:, 1::2] = w[:, 0::2]
    return rh


def _swiz(w):
    # (D, n) -> (128, NC, n) partition-major for contiguous DMA lines
    n = w.shape[1]
    return np.ascontiguousarray(
        w.reshape(NC, 128, n).transpose(1, 0, 2)
    ).astype(NPBF)


def make_in_maps(query, key, value, rot_pos_emb, q_kernel, k_kernel, v_kernel, o_kernel, L=L_FULL):
    f = np.asarray(rot_pos_emb, np.float32)
    cosT = np.cos(f).T.astype(np.float32)  # (32, L)
    sinT = np.sin(f).T.astype(np.float32)
    cosq = np.ones((128, L), np.float32)
    cosq[0:32] = cosT
    cosq[64:96] = cosT
    # sin table in qc row layout (rot rows of each head slot); pass rows unused
    sinr = np.zeros((128, L), np.float32)
    sinr[0:32] = sinT
    sinr[64:96] = sinT

    in_maps = []
    for core in range(NCORES):
        b, grp = divmod(core, NCORES // B)
        hs = list(range(grp * HPC, (grp + 1) * HPC))
        m = {"cosq": cosq, "sinr": sinr}
        for nm, x in (("xq", query), ("xk", key), ("xv", value)):
            xt = np.asarray(x[b], np.float32).T  # (D, L)
            m[nm] = _swiz(xt)
        for nm, wk in (("q", q_kernel), ("k", k_kernel)):
            wk = np.asarray(wk, np.float32)[:, hs, :]  # (D, 4, 64)
            m[f"w{nm}a0"] = _swiz(wk[:, 0:2].reshape(D, 128))
            m[f"w{nm}a1"] = _swiz(wk[:, 2:4].reshape(D, 128))
            m[f"w{nm}h"] = _swiz(
                np.concatenate([_rot_half_weights(wk[:, j, :ROT]) for j in range(4)], axis=1)
            )
        vk = np.asarray(v_kernel, np.float32)[:, hs, :]
        m["wv"] = _swiz(vk.reshape(D, 256))
        ok = np.asarray(o_kernel, np.float32)[hs]  # (4, 64, D)
        m["wo2"] = np.ascontiguousarray(ok.reshape(2, 128, D)).astype(NPBF)
        in_maps.append(m)
    return in_maps


_CACHED = {}


def kernel(query, key, value, rot_pos_emb, q_kernel, k_kernel, v_kernel, o_kernel):
    from concourse.bass_utils import run_bass_kernel_spmd

    if "nc" not in _CACHED:
        _CACHED["nc"] = build_nc(L_FULL)
    nc = _CACHED["nc"]
    in_maps = make_in_maps(
        query, key, value, rot_pos_emb, q_kernel, k_kernel, v_kernel, o_kernel
    )
    res = run_bass_kernel_spmd(nc, in_maps, core_ids=list(range(NCORES)))
    outs = res.results
    full = np.zeros((B, L_FULL, D), np.float32)
    for core in range(NCORES):
        b = core // (NCORES // B)
        full[b] += outs[core]["out"]
    return full
